# revision 21
# baseline (speedup 1.0000x reference)
"""Trainium2 Bass kernel for CycleWiseSelfAttention.

Problem: B=8, C=16, S=512, E=256 (fp32)
    q = relu(query @ Wq[c] + bq[c]) * E**-0.5
    k = relu(key   @ Wk[c] + bk[c])
    v = relu(value @ Wv[c] + bv[c])
    out = softmax(q @ k^T, axis=-1) @ v        (per (b, c) pair)

Sharding: cycle-parallel across 8 cores (2 cycles per core, all 8 batches).
Each core handles 16 independent (b, c) attention problems; per-cycle weights
go only to their owning core. No collectives.

Default build ("dr8"): all five matmul groups run as fp8e4 (e4m3) DoubleRow
matmuls — each instruction contracts K=256 (two 128-row halves packed per PE
cell), halving the PE streaming cycles vs fp16.  PSUM is one persistent
8-bank tensor with fixed per-pair roles:
    banks 0-1: q proj (f0,f1)  -> reused by scores t0,t1
    banks 2-3: k proj (f0,f1)  -> reused by scores t2,t3
    banks 4-7: v proj (t0..t3) -> reused by attn-out s0..s3
Fixed roles let every drain be one big cross-bank op:
    qrelu/krelu: one vector op over 2 banks each -> fp8 SBUF
    vrelu:       one vector op over 4 banks      -> fp8 SBUF
    exp:         ONE scalar activation over banks 0-3 (2048 el/partition),
                 with the softmax scale E**-0.5 and a -2 bias folded in
                 (softmax-invariant shift keeps fp8 exp outputs in range)
    norm:        scalar activation Copy with per-partition scale = 1/denom
The denominator comes from ones-columns appended to v (cols 256.. of a
272-wide v tile; 272 keeps the DoubleRow rhs stride a multiple of 16), so
attn-out computes [out_unnorm | denom] in one pass.

The f16 path (old baseline) is kept for the with_bias fallback and A/B runs.
"""

import numpy as np

B, C, S, E = 8, 16, 512, 256
N_CORES = 8
CYC = C // N_CORES          # cycles per core = 2
PAIRS_FULL = B * CYC        # (b, c) pairs per core = 16
P = 128
ECH = E // P                # e/f chunks = 2
SCH = S // P                # s/t chunks = 4
VF = 272                    # v free dim: 256 data + 16 ones cols (stride %16==0)
SCALE = float(E) ** -0.5
EXP_BIAS = -2.0             # softmax-invariant shift; keeps fp8 exp in range
MM_DTYPE = "dr8"


def _build_dr(pairs=PAIRS_FULL, proj_dtype="f8", attn_dtype="f8"):
    """Pipelined build (no bias support).

    proj_dtype="f16" keeps the three projection matmuls (and their DRAM
    inputs/weights) in fp16 for accuracy; attn_dtype picks fp8 DoubleRow
    ("f8") or plain fp16 ("f16") for the two attention matmul groups.
    """
    import concourse.bass as bass  # noqa: F401
    import concourse.bacc as bacc
    import concourse.tile as tile
    from concourse import mybir
    from contextlib import ExitStack

    f32 = mybir.dt.float32
    f8 = mybir.dt.float8e4
    f16 = mybir.dt.float16
    pdt = f8 if proj_dtype == "f8" else f16
    adt = f8 if attn_dtype == "f8" else f16
    vf = VF if attn_dtype == "f8" else E + 2
    DR = mybir.MatmulPerfMode.DoubleRow

    nc = bacc.Bacc("TRN2", target_bir_lowering=False, debug=False,
                   num_devices=N_CORES)

    qt = nc.dram_tensor("qt", [pairs, E, S], pdt, kind="ExternalInput").ap()
    kt = nc.dram_tensor("kt", [pairs, E, S], pdt, kind="ExternalInput").ap()
    vt = nc.dram_tensor("vt", [pairs, E, S], pdt, kind="ExternalInput").ap()
    wq = nc.dram_tensor("wq", [CYC, E, E], pdt, kind="ExternalInput").ap()
    wk = nc.dram_tensor("wk", [CYC, E, E], pdt, kind="ExternalInput").ap()
    wv = nc.dram_tensor("wv", [CYC, E, E], pdt, kind="ExternalInput").ap()
    out = nc.dram_tensor("out", [pairs, S, E], f32, kind="ExternalOutput").ap()

    Relu = mybir.ActivationFunctionType.Relu
    Exp = mybir.ActivationFunctionType.Exp
    Copy = mybir.ActivationFunctionType.Copy
    MAX = mybir.AluOpType.max
    MULT = mybir.AluOpType.mult

    def mm(ps, lhsT, rhs, start=True, stop=True):
        nc.tensor.matmul(ps, lhsT, rhs, start=start, stop=stop, perf_mode=DR)

    # engine per drain site: "v" = vector, "s" = scalar
    # keep Scalar's in-order queue clear of proj drains so the chain-critical
    # exp ops issue as early as possible; norms are tail work
    ENG_QRELU = "v"
    ENG_KRELU = "s"
    ENG_VRELU = "v"
    ENG_NORM = ("s", "v", "v", "v")   # per s-chunk

    def relu_drain(eng, dst, src):
        if eng == "v":
            nc.vector.tensor_scalar(dst, src, 0.0, None, MAX)
        else:
            nc.scalar.activation(dst, src, Relu)

    with tile.TileContext(nc) as tc, ExitStack() as ctx:
        wpool = ctx.enter_context(tc.tile_pool(name="w", bufs=1))
        inp = ctx.enter_context(tc.tile_pool(name="inp", bufs=2))
        proj = ctx.enter_context(tc.tile_pool(name="proj", bufs=2))
        expp = ctx.enter_context(tc.tile_pool(name="expp", bufs=2))
        outp = ctx.enter_context(tc.tile_pool(name="outp", bufs=2))
        dpool = ctx.enter_context(tc.tile_pool(name="dinv", bufs=8))
        # Two PSUM streams ("ps0"/"ps1"), each 2 bufs x 2 banks = 4 banks;
        # pairs alternate streams so stalls in one are covered by the other.
        psp = ctx.enter_context(tc.tile_pool(name="ps", bufs=2, space="PSUM"))

        ebias = wpool.tile([P, 1], f32, tag="ebias")
        nc.gpsimd.memset(ebias[:], EXP_BIAS)

        # persistent v tiles (2, alternating per pair); ones cols preset
        vs_t = []
        for i in range(2):
            t = wpool.tile([P, SCH, vf], adt, tag=f"vs{i}")
            nc.gpsimd.memset(t[:], 1.0)
            vs_t.append(t)

        # persistent weights [128, ech, E] per (proj, cycle), scalar ring.
        # Only cycle-0 weights load upfront (pairs are cc-major, so cycle-1
        # weights are not needed until halfway); cycle-1 loads are emitted
        # after the first couple to keep them off the startup critical path.
        wsrc = {"q": wq, "k": wk, "v": wv}
        wt = {}
        for name in ("q", "k", "v"):
            for cc in range(CYC):
                wt[name, cc] = wpool.tile([P, ECH, E], pdt, tag=f"w{name}{cc}",
                                          name=f"w{name}{cc}")
        def load_weights(cc, ring):
            for name in ("q", "k", "v"):
                ring.dma_start(
                    out=wt[name, cc][:],
                    in_=wsrc[name][cc].rearrange("(ch p) f -> p ch f", p=P))
        load_weights(0, nc.scalar)
        for cc in range(1, CYC):
            load_weights(cc, nc.gpsimd)

        # Dual-stream emission: pairs 2i and 2i+1 advance in lockstep with
        # separate 4-bank PSUM sets ("psA"/"psB"), interleaved stage by
        # stage, so either stream's dependency stalls are covered by the
        # other stream's ready work.
        def ps_tile(p_idx):
            return psp.tile([P, 2, S], f32, tag=f"ps{p_idx % 2}",
                            name=f"psT{p_idx % 2}")

        def st_load(pb, PB):
            qT_in = inp.tile([P, PB, ECH, S], pdt, tag="qT_in")
            kT_in = inp.tile([P, PB, ECH, S], pdt, tag="kT_in")
            vT_in = inp.tile([P, PB, ECH, S], pdt, tag="vT_in")
            for t, d in ((qT_in, qt), (kT_in, kt), (vT_in, vt)):
                nc.sync.dma_start(
                    out=t[:],
                    in_=d[pb : pb + PB].rearrange("pp (ch p) s -> p pp ch s", p=P))
            outb = outp.tile([P, PB, SCH, E], f32, tag="outs")
            return qT_in, kT_in, vT_in, outb

        def pair_cc(p_idx):
            return (p_idx * CYC) // pairs

        def st_qk(p_idx, sub, srct, wname, eng):
            cc = pair_cc(p_idx)
            T = ps_tile(p_idx)
            for f in range(ECH):
                fsl = slice(f * P, (f + 1) * P)
                if proj_dtype == "f8":
                    mm(T[:, f, :], wt[wname, cc][:, :, fsl], srct[:, sub])
                else:
                    for e in range(ECH):
                        nc.tensor.matmul(
                            T[:, f, :], wt[wname, cc][:, e, fsl],
                            srct[:, sub, e, :],
                            start=(e == 0), stop=(e == ECH - 1))
            dst = proj.tile([P, ECH, S], adt, tag=wname + "Ts",
                            name=wname + "Ts")
            relu_drain(eng, dst[:], T[:])
            return dst

        def st_v(p_idx, sub, vT_in):
            cc = pair_cc(p_idx)
            vs = vs_t[p_idx % 2]
            T = ps_tile(p_idx)
            for t in range(SCH):
                tsl = slice(t * P, (t + 1) * P)
                osl = slice((t % 2) * E, (t % 2) * E + E)
                if proj_dtype == "f8":
                    mm(T[:, t // 2, osl], vT_in[:, sub, :, tsl],
                       wt["v", cc][:])
                else:
                    for e in range(ECH):
                        nc.tensor.matmul(
                            T[:, t // 2, osl], vT_in[:, sub, e, tsl],
                            wt["v", cc][:, e, :],
                            start=(e == 0), stop=(e == ECH - 1))
            relu_drain(
                ENG_VRELU,
                vs[:, :, :E].rearrange("p (b c) x -> p b c x", b=2),
                T[:].rearrange("p b (c x) -> p b c x", x=E))
            return vs

        def st_sc(p_idx, g, qTs, kTs, expTs):
            T = ps_tile(p_idx)
            for i in range(2):
                t = 2 * g + i
                tsl = slice(t * P, (t + 1) * P)
                if attn_dtype == "f8":
                    mm(T[:, i, :], kTs[:, :, tsl], qTs[:])
                else:
                    for f in range(ECH):
                        nc.tensor.matmul(
                            T[:, i, :], kTs[:, f, tsl], qTs[:, f, :],
                            start=(f == 0), stop=(f == ECH - 1))
            nc.scalar.activation(expTs[:, 2 * g : 2 * g + 2, :], T[:], Exp,
                                 bias=ebias[:], scale=SCALE)

        def st_o(p_idx, sub, sg, expTs, vs, outb):
            T = ps_tile(p_idx)
            for si in range(2):
                s = sg * 2 + si
                ssl = slice(s * P, (s + 1) * P)
                if attn_dtype == "f8":
                    for i in range(2):
                        mm(T[:, si, :vf],
                           expTs[:, 2 * i : 2 * i + 2, ssl],
                           vs[:, 2 * i : 2 * i + 2, :],
                           start=(i == 0), stop=(i == 1))
                else:
                    for t in range(SCH):
                        nc.tensor.matmul(
                            T[:, si, :vf], expTs[:, t, ssl], vs[:, t, :],
                            start=(t == 0), stop=(t == SCH - 1))
            dinv = dpool.tile([P, 2], f32, tag="dinv")
            nc.vector.reciprocal(dinv[:], T[:, :, E : E + 1])
            for si in range(2):
                s = sg * 2 + si
                if ENG_NORM[s] == "s":
                    nc.scalar.activation(
                        outb[:, sub, s, :], T[:, si, :E], Copy,
                        scale=dinv[:, si : si + 1])
                else:
                    nc.vector.tensor_scalar(
                        outb[:, sub, s, :], T[:, si, :E],
                        dinv[:, si : si + 1], None, MULT)

        # single-pair first batches shrink the head (first matmul waits on a
        # 0.5 MB load, not 3 MB); single-pair last batches shrink the store
        # tail the same way
        if pairs >= 6 and pairs % 2 == 0:
            couples = [(0, 1), (1, 1)] \
                + [(pb, 2) for pb in range(2, pairs - 2, 2)] \
                + [(pairs - 2, 1), (pairs - 1, 1)]
        else:
            couples = [(pb, 1) for pb in range(pairs)]
        for pb, PB in couples:
            qT_in, kT_in, vT_in, outb = st_load(pb, PB)
            subs = list(range(PB))
            qTs = {}; kTs = {}; vsd = {}; expd = {}
            for sub in subs:
                qTs[sub] = st_qk(pb + sub, sub, qT_in, "q", ENG_QRELU)
            for sub in subs:
                kTs[sub] = st_qk(pb + sub, sub, kT_in, "k", ENG_KRELU)
            for sub in subs:
                vsd[sub] = st_v(pb + sub, sub, vT_in)
            for sub in subs:
                expd[sub] = expp.tile([P, SCH, S], adt, tag="expTs",
                                      name="expTs")
            for g in range(2):
                for sub in subs:
                    st_sc(pb + sub, g, qTs[sub], kTs[sub], expd[sub])
            for sg in range(2):
                for sub in subs:
                    st_o(pb + sub, sub, sg, expd[sub], vsd[sub], outb)
            nc.gpsimd.dma_start(
                out=out[pb : pb + PB].rearrange(
                    "pp (sch p) e -> p pp sch e", p=P),
                in_=outb[:])

    nc.compile()
    return nc


def _build_f16(pairs=PAIRS_FULL, with_bias=False, mm_dtype="f16"):
    """Old fp16/f32r build (supports bias); kept as fallback."""
    import concourse.bass as bass  # noqa: F401
    import concourse.bacc as bacc
    import concourse.tile as tile
    from concourse import mybir
    from contextlib import ExitStack

    f32 = mybir.dt.float32
    mmdt = {"f32r": mybir.dt.float32r, "f32": mybir.dt.float32,
            "f16": mybir.dt.float16, "bf16": mybir.dt.bfloat16}[mm_dtype]
    exp_bias = -2.0 if mm_dtype in ("f16", "bf16") else 0.0

    nc = bacc.Bacc("TRN2", target_bir_lowering=False, debug=False,
                   num_devices=N_CORES)

    qt = nc.dram_tensor("qt", [pairs, E, S], mmdt, kind="ExternalInput").ap()
    kt = nc.dram_tensor("kt", [pairs, E, S], mmdt, kind="ExternalInput").ap()
    vt = nc.dram_tensor("vt", [pairs, E, S], mmdt, kind="ExternalInput").ap()
    wq = nc.dram_tensor("wq", [CYC, E, E], mmdt, kind="ExternalInput").ap()
    wk = nc.dram_tensor("wk", [CYC, E, E], mmdt, kind="ExternalInput").ap()
    wv = nc.dram_tensor("wv", [CYC, E, E], mmdt, kind="ExternalInput").ap()
    if with_bias:
        bq = nc.dram_tensor("bq", [CYC, E], mmdt, kind="ExternalInput").ap()
        bk = nc.dram_tensor("bk", [CYC, E], mmdt, kind="ExternalInput").ap()
        bv = nc.dram_tensor("bv", [CYC, E], mmdt, kind="ExternalInput").ap()
    out = nc.dram_tensor("out", [pairs, S, E], f32, kind="ExternalOutput").ap()

    Relu = mybir.ActivationFunctionType.Relu  # noqa: F841
    Exp = mybir.ActivationFunctionType.Exp
    MAX = mybir.AluOpType.max
    MULT = mybir.AluOpType.mult

    with tile.TileContext(nc) as tc, ExitStack() as ctx:
        wpool = ctx.enter_context(tc.tile_pool(name="w", bufs=1))
        inp = ctx.enter_context(tc.tile_pool(name="inp", bufs=2))
        proj = ctx.enter_context(tc.tile_pool(name="proj", bufs=2))
        expp = ctx.enter_context(tc.tile_pool(name="expp", bufs=2))
        outp = ctx.enter_context(tc.tile_pool(name="outp", bufs=2))
        dpool = ctx.enter_context(tc.tile_pool(name="dinv", bufs=8))
        ps_qk = ctx.enter_context(tc.tile_pool(name="psqk", bufs=2, space="PSUM"))
        ps_sc = ctx.enter_context(tc.tile_pool(name="pssc", bufs=2, space="PSUM"))
        ps_v = ctx.enter_context(tc.tile_pool(name="psv", bufs=2, space="PSUM"))
        ps_o = ctx.enter_context(tc.tile_pool(name="pso", bufs=2, space="PSUM"))

        ones_col = wpool.tile([P, SCH, 2], f32, tag="ones_col")
        nc.gpsimd.memset(ones_col[:], 1.0)
        ebias_t = None
        if exp_bias != 0.0:
            ebias_t = wpool.tile([P, 1], f32, tag="ebias")
            nc.gpsimd.memset(ebias_t[:], exp_bias)

        wt = {}
        for cc in range(CYC):
            for name, wd in (("q", wq), ("k", wk), ("v", wv)):
                t = wpool.tile([P, ECH, E], mmdt, tag=f"w{name}{cc}")
                nc.scalar.dma_start(
                    out=t[:], in_=wd[cc].rearrange("(ch p) f -> p ch f", p=P))
                wt[name, cc] = t
        if with_bias:
            bt = {}
            for name, bd in (("q", bq), ("k", bk), ("v", bv)):
                for cc in range(CYC):
                    t = wpool.tile([1, E], mmdt, tag=f"b{name}{cc}")
                    nc.sync.dma_start(out=t[:], in_=bd[cc : cc + 1, :])
                    bt[name, cc] = t
            ones_f32 = wpool.tile([1, S], f32, tag="ones_f32")
            nc.gpsimd.memset(ones_f32[:], 1.0)
            ones_row = wpool.tile([1, S], mmdt, tag="ones")
            nc.vector.tensor_copy(ones_row[:], ones_f32[:])

        if pairs >= 4 and pairs % 2 == 0:
            batches = [(0, 1)] + [(i, 2) for i in range(1, pairs - 1, 2)] \
                + [(pairs - 1, 1)]
        else:
            batches = [(i, 1) for i in range(pairs)]
        for pb, PB in batches:
            qT_in = inp.tile([P, PB, ECH, S], mmdt, tag="qT_in")
            kT_in = inp.tile([P, PB, ECH, S], mmdt, tag="kT_in")
            vT_in = inp.tile([P, PB, ECH, S], mmdt, tag="vT_in")
            for t, d in ((qT_in, qt), (kT_in, kt), (vT_in, vt)):
                nc.sync.dma_start(
                    out=t[:],
                    in_=d[pb : pb + PB].rearrange("pp (ch p) s -> p pp ch s", p=P))
            outb = outp.tile([P, PB, SCH, E], f32, tag="outs")

            def stage_proj_qk(sub):
                p_idx = pb + sub
                cc = (p_idx * CYC) // pairs
                qTs = proj.tile([P, ECH, S], mmdt, tag="qTs")
                kTs = proj.tile([P, ECH, S], mmdt, tag="kTs")
                for name, srct, dst, scl in (("q", qT_in, qTs, SCALE),
                                             ("k", kT_in, kTs, 1.0)):
                    w = wt[name, cc]
                    for f in range(ECH):
                        ps = ps_qk.tile([P, S], f32, tag="ps_qk")
                        fsl = slice(f * P, (f + 1) * P)
                        for e in range(ECH):
                            nc.tensor.matmul(
                                ps[:], w[:, e, fsl], srct[:, sub, e, :],
                                start=(e == 0),
                                stop=(e == ECH - 1 and not with_bias))
                        if with_bias:
                            nc.tensor.matmul(
                                ps[:], bt[name, cc][:, fsl], ones_row[:],
                                start=False, stop=True)
                        if name == "q":
                            nc.scalar.activation(dst[:, f, :], ps[:], Relu,
                                                 scale=scl)
                        else:
                            nc.vector.tensor_scalar(
                                dst[:, f, :], ps[:], 0.0, None, MAX)
                return qTs, kTs

            def stage_v(sub):
                p_idx = pb + sub
                cc = (p_idx * CYC) // pairs
                vs = proj.tile([P, SCH, E + 2], mmdt, tag="vs")
                nc.vector.tensor_copy(vs[:, :, E : E + 2], ones_col[:])
                w = wt["v", cc]
                for t in range(SCH):
                    ps = ps_v.tile([P, E + 2], f32, tag="ps_v")
                    tsl = slice(t * P, (t + 1) * P)
                    for e in range(ECH):
                        nc.tensor.matmul(
                            ps[:, :E], vT_in[:, sub, e, tsl], w[:, e, :],
                            start=(e == 0),
                            stop=(e == ECH - 1 and not with_bias))
                    if with_bias:
                        nc.tensor.matmul(
                            ps[:, :E], ones_row[:, tsl], bt["v", cc][:],
                            start=False, stop=True)
                    nc.vector.tensor_scalar(
                        vs[:, t, :E], ps[:, :E], 0.0, None, MAX)
                return vs

            def stage_scores(sub, qTs, kTs):
                expTs = expp.tile([P, SCH, S], mmdt, tag="expTs")
                for t in range(SCH):
                    ps = ps_sc.tile([P, S], f32, tag="ps_sc")
                    tsl = slice(t * P, (t + 1) * P)
                    for f in range(ECH):
                        nc.tensor.matmul(
                            ps[:], kTs[:, f, tsl], qTs[:, f, :],
                            start=(f == 0), stop=(f == ECH - 1))
                    if ebias_t is None:
                        nc.scalar.activation(expTs[:, t, :], ps[:], Exp)
                    else:
                        nc.scalar.activation(expTs[:, t, :], ps[:], Exp,
                                             bias=ebias_t[:])
                return expTs

            def stage_out(sub, expTs, vs):
                for s in range(SCH):
                    ps = ps_o.tile([P, E + 2], f32, tag="ps_o")
                    ssl = slice(s * P, (s + 1) * P)
                    for t in range(SCH):
                        nc.tensor.matmul(
                            ps[:], expTs[:, t, ssl], vs[:, t, :],
                            start=(t == 0), stop=(t == SCH - 1))
                    dinv = dpool.tile([P, 1], f32, tag="dinv")
                    nc.vector.reciprocal(dinv[:], ps[:, E : E + 1])
                    nc.vector.tensor_scalar(
                        outb[:, sub, s, :], ps[:, :E], dinv[:], None, MULT)

            for sub in range(PB):
                qTs, kTs = stage_proj_qk(sub)
                vs = stage_v(sub)
                expTs = stage_scores(sub, qTs, kTs)
                stage_out(sub, expTs, vs)

            for sub in range(PB):
                nc.scalar.dma_start(
                    out=out[pb + sub].rearrange("(sch p) e -> p sch e", p=P),
                    in_=outb[:, sub])

    nc.compile()
    return nc


_BUILT = {}


def _get_built(pairs=PAIRS_FULL, with_bias=False, mm_dtype=MM_DTYPE):
    key = (pairs, with_bias, mm_dtype)
    if key not in _BUILT:
        if mm_dtype == "dr8":
            assert not with_bias
            _BUILT[key] = _build_dr(pairs, proj_dtype="f8")
        elif mm_dtype == "mix8":
            assert not with_bias
            _BUILT[key] = _build_dr(pairs, proj_dtype="f16", attn_dtype="f8")
        elif mm_dtype == "f16v4":
            assert not with_bias
            _BUILT[key] = _build_dr(pairs, proj_dtype="f16", attn_dtype="f16")
        else:
            _BUILT[key] = _build_f16(pairs, with_bias, mm_dtype)
    return _BUILT[key]


def _round_fp32r(a):
    a = np.ascontiguousarray(a, dtype=np.float32)
    u = a.view(np.uint32).copy()
    u += np.uint32(0x800)
    u &= np.uint32(0xFFFFF000)
    return u.view(np.float32)


def _converter(mm_dtype):
    if mm_dtype == "f32r":
        return _round_fp32r
    if mm_dtype == "f16":
        return lambda x: np.ascontiguousarray(x, np.float16)
    if mm_dtype == "bf16":
        import ml_dtypes
        return lambda x: np.ascontiguousarray(
            np.asarray(x, np.float32).astype(ml_dtypes.bfloat16))
    if mm_dtype == "dr8":
        import ml_dtypes
        return lambda x: np.ascontiguousarray(
            np.asarray(x, np.float32).astype(ml_dtypes.float8_e4m3))
    if mm_dtype in ("mix8", "f16v4"):
        return lambda x: np.ascontiguousarray(x, np.float16)
    return lambda x: np.ascontiguousarray(x, np.float32)


def _shard_inputs(query, key, value, wq, wk, wv, bq, bk, bv, with_bias,
                  mm_dtype=MM_DTYPE):
    """Per-core input maps. Core m owns cycles [m*CYC, (m+1)*CYC)."""
    r = _converter(mm_dtype)
    in_maps = []
    for m in range(N_CORES):
        cs = slice(m * CYC, (m + 1) * CYC)
        im = {
            # [B, CYC, S, E] -> [pairs(cc-major), E, S] (host-side transpose)
            "qt": r(query[:, cs].transpose(1, 0, 3, 2)).reshape(PAIRS_FULL, E, S),
            "kt": r(key[:, cs].transpose(1, 0, 3, 2)).reshape(PAIRS_FULL, E, S),
            "vt": r(value[:, cs].transpose(1, 0, 3, 2)).reshape(PAIRS_FULL, E, S),
            "wq": r(wq[cs]),
            "wk": r(wk[cs]),
            "wv": r(wv[cs]),
        }
        if with_bias:
            im["bq"] = r(bq[cs, 0])
            im["bk"] = r(bk[cs, 0])
            im["bv"] = r(bv[cs, 0])
        in_maps.append(im)
    return in_maps


def kernel(**inputs):
    from concourse.bass_utils import run_bass_kernel_spmd

    query = np.asarray(inputs["query"], dtype=np.float32)
    key = np.asarray(inputs["key"], dtype=np.float32)
    value = np.asarray(inputs["value"], dtype=np.float32)
    wq = np.asarray(inputs["q_proj_weight"], dtype=np.float32)
    wk = np.asarray(inputs["k_proj_weight"], dtype=np.float32)
    wv = np.asarray(inputs["v_proj_weight"], dtype=np.float32)
    bq = np.asarray(inputs["q_proj_bias"], dtype=np.float32)
    bk = np.asarray(inputs["k_proj_bias"], dtype=np.float32)
    bv = np.asarray(inputs["v_proj_bias"], dtype=np.float32)

    with_bias = bool(np.any(bq) or np.any(bk) or np.any(bv))
    mm_dtype = "f16" if with_bias else MM_DTYPE
    nc = _get_built(PAIRS_FULL, with_bias, mm_dtype)
    in_maps = _shard_inputs(query, key, value, wq, wk, wv, bq, bk, bv,
                            with_bias, mm_dtype)

    res = None
    for attempt in range(3):
        try:
            res = run_bass_kernel_spmd(nc, in_maps, list(range(N_CORES)))
            break
        except Exception:
            if attempt == 2:
                raise
    out = np.empty((B, C, S, E), dtype=np.float32)
    for m in range(N_CORES):
        o = res.results[m]["out"].reshape(CYC, B, S, E)
        out[:, m * CYC : (m + 1) * CYC] = o.transpose(1, 0, 2, 3)
    return out


if __name__ == "__main__":
    rng = np.random.default_rng(0)
    ins = {
        "query": rng.standard_normal((B, C, S, E), dtype=np.float32),
        "key": rng.standard_normal((B, C, S, E), dtype=np.float32),
        "value": rng.standard_normal((B, C, S, E), dtype=np.float32),
        "q_proj_weight": rng.standard_normal((C, E, E), dtype=np.float32) * 0.0625,
        "k_proj_weight": rng.standard_normal((C, E, E), dtype=np.float32) * 0.0625,
        "v_proj_weight": rng.standard_normal((C, E, E), dtype=np.float32) * 0.0625,
        "q_proj_bias": np.zeros((C, 1, E), np.float32),
        "k_proj_bias": np.zeros((C, 1, E), np.float32),
        "v_proj_bias": np.zeros((C, 1, E), np.float32),
    }
    o = kernel(**ins)
    print("out", o.shape, o.dtype, float(np.abs(o).max()))


# revision 22
# speedup vs baseline: 1.1741x; 1.1741x over previous
"""Trainium2 Bass kernel for CycleWiseSelfAttention.

Problem: B=8, C=16, S=512, E=256 (fp32)
    q = relu(query @ Wq[c] + bq[c]) * E**-0.5
    k = relu(key   @ Wk[c] + bk[c])
    v = relu(value @ Wv[c] + bv[c])
    out = softmax(q @ k^T, axis=-1) @ v        (per (b, c) pair)

Sharding: cycle-parallel across 8 cores (2 cycles per core, all 8 batches).
Each core handles 16 independent (b, c) attention problems; per-cycle weights
go only to their owning core. No collectives.

Default build ("dr8"): all five matmul groups run as fp8e4 (e4m3) DoubleRow
matmuls — each instruction contracts K=256 (two 128-row halves packed per PE
cell), halving the PE streaming cycles vs fp16.  PSUM is one persistent
8-bank tensor with fixed per-pair roles:
    banks 0-1: q proj (f0,f1)  -> reused by scores t0,t1
    banks 2-3: k proj (f0,f1)  -> reused by scores t2,t3
    banks 4-7: v proj (t0..t3) -> reused by attn-out s0..s3
Fixed roles let every drain be one big cross-bank op:
    qrelu/krelu: one vector op over 2 banks each -> fp8 SBUF
    vrelu:       one vector op over 4 banks      -> fp8 SBUF
    exp:         ONE scalar activation over banks 0-3 (2048 el/partition),
                 with the softmax scale E**-0.5 and a -2 bias folded in
                 (softmax-invariant shift keeps fp8 exp outputs in range)
    norm:        scalar activation Copy with per-partition scale = 1/denom
The denominator comes from ones-columns appended to v (cols 256.. of a
272-wide v tile; 272 keeps the DoubleRow rhs stride a multiple of 16), so
attn-out computes [out_unnorm | denom] in one pass.

The f16 path (old baseline) is kept for the with_bias fallback and A/B runs.
"""

import numpy as np

B, C, S, E = 8, 16, 512, 256
N_CORES = 8
CYC = C // N_CORES          # cycles per core = 2
PAIRS_FULL = B * CYC        # (b, c) pairs per core = 16
P = 128
ECH = E // P                # e/f chunks = 2
SCH = S // P                # s/t chunks = 4
VF = 272                    # v free dim: 256 data + 16 ones cols (stride %16==0)
SCALE = float(E) ** -0.5
EXP_BIAS = -2.0             # softmax-invariant shift; keeps fp8 exp in range
MM_DTYPE = "dr8"


def _build_dr(pairs=PAIRS_FULL, proj_dtype="f8", attn_dtype="f8"):
    """Pipelined build (no bias support).

    proj_dtype="f16" keeps the three projection matmuls (and their DRAM
    inputs/weights) in fp16 for accuracy; attn_dtype picks fp8 DoubleRow
    ("f8") or plain fp16 ("f16") for the two attention matmul groups.
    """
    import concourse.bass as bass  # noqa: F401
    import concourse.bacc as bacc
    import concourse.tile as tile
    from concourse import mybir
    from contextlib import ExitStack

    f32 = mybir.dt.float32
    f8 = mybir.dt.float8e4
    f16 = mybir.dt.float16
    pdt = f8 if proj_dtype == "f8" else f16
    adt = f8 if attn_dtype == "f8" else f16
    vf = VF if attn_dtype == "f8" else E + 2
    DR = mybir.MatmulPerfMode.DoubleRow

    nc = bacc.Bacc("TRN2", target_bir_lowering=False, debug=False,
                   num_devices=N_CORES)

    qt = nc.dram_tensor("qt", [pairs, E, S], pdt, kind="ExternalInput").ap()
    kt = nc.dram_tensor("kt", [pairs, E, S], pdt, kind="ExternalInput").ap()
    vt = nc.dram_tensor("vt", [pairs, E, S], pdt, kind="ExternalInput").ap()
    wq = nc.dram_tensor("wq", [CYC, E, E], pdt, kind="ExternalInput").ap()
    wk = nc.dram_tensor("wk", [CYC, E, E], pdt, kind="ExternalInput").ap()
    wv = nc.dram_tensor("wv", [CYC, E, E], pdt, kind="ExternalInput").ap()
    out = nc.dram_tensor("out", [pairs, S, E], f32, kind="ExternalOutput").ap()

    Relu = mybir.ActivationFunctionType.Relu
    Exp = mybir.ActivationFunctionType.Exp
    Copy = mybir.ActivationFunctionType.Copy
    MAX = mybir.AluOpType.max
    MULT = mybir.AluOpType.mult

    def mm(ps, lhsT, rhs, start=True, stop=True):
        nc.tensor.matmul(ps, lhsT, rhs, start=start, stop=stop, perf_mode=DR)

    # engine per drain site: "v" = vector, "s" = scalar
    # keep Scalar's in-order queue clear of proj drains so the chain-critical
    # exp ops issue as early as possible; norms are tail work
    ENG_QRELU = "v"
    ENG_KRELU = "s"
    ENG_VRELU = "v"
    ENG_NORM = ("s", "v", "v", "v")   # per s-chunk

    def relu_drain(eng, dst, src):
        if eng == "v":
            nc.vector.tensor_scalar(dst, src, 0.0, None, MAX)
        else:
            nc.scalar.activation(dst, src, Relu)

    with tile.TileContext(nc) as tc, ExitStack() as ctx:
        wpool = ctx.enter_context(tc.tile_pool(name="w", bufs=1))
        inp = ctx.enter_context(tc.tile_pool(name="inp", bufs=2))
        proj = ctx.enter_context(tc.tile_pool(name="proj", bufs=2))
        expp = ctx.enter_context(tc.tile_pool(name="expp", bufs=2))
        outp = ctx.enter_context(tc.tile_pool(name="outp", bufs=2))
        dpool = ctx.enter_context(tc.tile_pool(name="dinv", bufs=8))
        # Two PSUM streams ("ps0"/"ps1"), each 2 bufs x 2 banks = 4 banks;
        # pairs alternate streams so stalls in one are covered by the other.
        psp = ctx.enter_context(tc.tile_pool(name="ps", bufs=2, space="PSUM"))

        ebias = wpool.tile([P, 1], f32, tag="ebias")
        nc.gpsimd.memset(ebias[:], EXP_BIAS)

        # persistent v tiles (2, alternating per pair); ones cols preset
        vs_t = []
        for i in range(2):
            t = wpool.tile([P, SCH, vf], adt, tag=f"vs{i}")
            nc.gpsimd.memset(t[:], 1.0)
            vs_t.append(t)

        # persistent weights [128, ech, E] per (proj, cycle), scalar ring.
        # Only cycle-0 weights load upfront (pairs are cc-major, so cycle-1
        # weights are not needed until halfway); cycle-1 loads are emitted
        # after the first couple to keep them off the startup critical path.
        wsrc = {"q": wq, "k": wk, "v": wv}
        wt = {}
        for name in ("q", "k", "v"):
            for cc in range(CYC):
                wt[name, cc] = wpool.tile([P, ECH, E], pdt, tag=f"w{name}{cc}",
                                          name=f"w{name}{cc}")
        def load_weights(cc, ring):
            for name in ("q", "k", "v"):
                ring.dma_start(
                    out=wt[name, cc][:],
                    in_=wsrc[name][cc].rearrange("(ch p) f -> p ch f", p=P))
        for cc in range(CYC):
            load_weights(cc, nc.scalar)

        # Dual-stream emission: pairs 2i and 2i+1 advance in lockstep with
        # separate 4-bank PSUM sets ("psA"/"psB"), interleaved stage by
        # stage, so either stream's dependency stalls are covered by the
        # other stream's ready work.
        def ps_tile(p_idx):
            return psp.tile([P, 2, S], f32, tag=f"ps{p_idx % 2}",
                            name=f"psT{p_idx % 2}")

        def st_load(pb, PB):
            qT_in = inp.tile([P, PB, ECH, S], pdt, tag="qT_in")
            kT_in = inp.tile([P, PB, ECH, S], pdt, tag="kT_in")
            vT_in = inp.tile([P, PB, ECH, S], pdt, tag="vT_in")
            for t, d in ((qT_in, qt), (kT_in, kt), (vT_in, vt)):
                nc.sync.dma_start(
                    out=t[:],
                    in_=d[pb : pb + PB].rearrange("pp (ch p) s -> p pp ch s", p=P))
            outb = outp.tile([P, PB, SCH, E], f32, tag="outs")
            return qT_in, kT_in, vT_in, outb

        def pair_cc(p_idx):
            return (p_idx * CYC) // pairs

        def st_qk(p_idx, sub, srct, wname, eng):
            cc = pair_cc(p_idx)
            T = ps_tile(p_idx)
            for f in range(ECH):
                fsl = slice(f * P, (f + 1) * P)
                if proj_dtype == "f8":
                    mm(T[:, f, :], wt[wname, cc][:, :, fsl], srct[:, sub])
                else:
                    for e in range(ECH):
                        nc.tensor.matmul(
                            T[:, f, :], wt[wname, cc][:, e, fsl],
                            srct[:, sub, e, :],
                            start=(e == 0), stop=(e == ECH - 1))
            dst = proj.tile([P, ECH, S], adt, tag=wname + "Ts",
                            name=wname + "Ts")
            relu_drain(eng, dst[:], T[:])
            return dst

        def st_v(p_idx, sub, vT_in):
            cc = pair_cc(p_idx)
            vs = vs_t[p_idx % 2]
            T = ps_tile(p_idx)
            for t in range(SCH):
                tsl = slice(t * P, (t + 1) * P)
                osl = slice((t % 2) * E, (t % 2) * E + E)
                if proj_dtype == "f8":
                    mm(T[:, t // 2, osl], vT_in[:, sub, :, tsl],
                       wt["v", cc][:])
                else:
                    for e in range(ECH):
                        nc.tensor.matmul(
                            T[:, t // 2, osl], vT_in[:, sub, e, tsl],
                            wt["v", cc][:, e, :],
                            start=(e == 0), stop=(e == ECH - 1))
            relu_drain(
                ENG_VRELU,
                vs[:, :, :E].rearrange("p (b c) x -> p b c x", b=2),
                T[:].rearrange("p b (c x) -> p b c x", x=E))
            return vs

        def st_sc(p_idx, g, qTs, kTs, expTs):
            T = ps_tile(p_idx)
            for i in range(2):
                t = 2 * g + i
                tsl = slice(t * P, (t + 1) * P)
                if attn_dtype == "f8":
                    mm(T[:, i, :], kTs[:, :, tsl], qTs[:])
                else:
                    for f in range(ECH):
                        nc.tensor.matmul(
                            T[:, i, :], kTs[:, f, tsl], qTs[:, f, :],
                            start=(f == 0), stop=(f == ECH - 1))
            nc.scalar.activation(expTs[:, 2 * g : 2 * g + 2, :], T[:], Exp,
                                 bias=ebias[:], scale=SCALE)

        def st_o(p_idx, sub, sg, expTs, vs, outb):
            T = ps_tile(p_idx)
            for si in range(2):
                s = sg * 2 + si
                ssl = slice(s * P, (s + 1) * P)
                if attn_dtype == "f8":
                    for i in range(2):
                        mm(T[:, si, :vf],
                           expTs[:, 2 * i : 2 * i + 2, ssl],
                           vs[:, 2 * i : 2 * i + 2, :],
                           start=(i == 0), stop=(i == 1))
                else:
                    for t in range(SCH):
                        nc.tensor.matmul(
                            T[:, si, :vf], expTs[:, t, ssl], vs[:, t, :],
                            start=(t == 0), stop=(t == SCH - 1))
            dinv = dpool.tile([P, 2], f32, tag="dinv")
            nc.vector.reciprocal(dinv[:], T[:, :, E : E + 1])
            for si in range(2):
                s = sg * 2 + si
                if ENG_NORM[s] == "s":
                    nc.scalar.activation(
                        outb[:, sub, s, :], T[:, si, :E], Copy,
                        scale=dinv[:, si : si + 1])
                else:
                    nc.vector.tensor_scalar(
                        outb[:, sub, s, :], T[:, si, :E],
                        dinv[:, si : si + 1], None, MULT)

        # single-pair first batches shrink the head (first matmul waits on a
        # 0.5 MB load, not 3 MB); single-pair last batches shrink the store
        # tail the same way
        if pairs >= 6 and pairs % 2 == 0:
            couples = [(0, 1), (1, 1)] \
                + [(pb, 2) for pb in range(2, pairs - 2, 2)] \
                + [(pairs - 2, 1), (pairs - 1, 1)]
        else:
            couples = [(pb, 1) for pb in range(pairs)]
        for pb, PB in couples:
            qT_in, kT_in, vT_in, outb = st_load(pb, PB)
            subs = list(range(PB))
            qTs = {}; kTs = {}; vsd = {}; expd = {}
            for sub in subs:
                qTs[sub] = st_qk(pb + sub, sub, qT_in, "q", ENG_QRELU)
            for sub in subs:
                kTs[sub] = st_qk(pb + sub, sub, kT_in, "k", ENG_KRELU)
            for sub in subs:
                vsd[sub] = st_v(pb + sub, sub, vT_in)
            for sub in subs:
                expd[sub] = expp.tile([P, SCH, S], adt, tag="expTs",
                                      name="expTs")
            for g in range(2):
                for sub in subs:
                    st_sc(pb + sub, g, qTs[sub], kTs[sub], expd[sub])
            for sg in range(2):
                for sub in subs:
                    st_o(pb + sub, sub, sg, expd[sub], vsd[sub], outb)
            nc.gpsimd.dma_start(
                out=out[pb : pb + PB].rearrange(
                    "pp (sch p) e -> p pp sch e", p=P),
                in_=outb[:])

    nc.compile()
    return nc


def _build_f16(pairs=PAIRS_FULL, with_bias=False, mm_dtype="f16"):
    """Old fp16/f32r build (supports bias); kept as fallback."""
    import concourse.bass as bass  # noqa: F401
    import concourse.bacc as bacc
    import concourse.tile as tile
    from concourse import mybir
    from contextlib import ExitStack

    f32 = mybir.dt.float32
    mmdt = {"f32r": mybir.dt.float32r, "f32": mybir.dt.float32,
            "f16": mybir.dt.float16, "bf16": mybir.dt.bfloat16}[mm_dtype]
    exp_bias = -2.0 if mm_dtype in ("f16", "bf16") else 0.0

    nc = bacc.Bacc("TRN2", target_bir_lowering=False, debug=False,
                   num_devices=N_CORES)

    qt = nc.dram_tensor("qt", [pairs, E, S], mmdt, kind="ExternalInput").ap()
    kt = nc.dram_tensor("kt", [pairs, E, S], mmdt, kind="ExternalInput").ap()
    vt = nc.dram_tensor("vt", [pairs, E, S], mmdt, kind="ExternalInput").ap()
    wq = nc.dram_tensor("wq", [CYC, E, E], mmdt, kind="ExternalInput").ap()
    wk = nc.dram_tensor("wk", [CYC, E, E], mmdt, kind="ExternalInput").ap()
    wv = nc.dram_tensor("wv", [CYC, E, E], mmdt, kind="ExternalInput").ap()
    if with_bias:
        bq = nc.dram_tensor("bq", [CYC, E], mmdt, kind="ExternalInput").ap()
        bk = nc.dram_tensor("bk", [CYC, E], mmdt, kind="ExternalInput").ap()
        bv = nc.dram_tensor("bv", [CYC, E], mmdt, kind="ExternalInput").ap()
    out = nc.dram_tensor("out", [pairs, S, E], f32, kind="ExternalOutput").ap()

    Relu = mybir.ActivationFunctionType.Relu  # noqa: F841
    Exp = mybir.ActivationFunctionType.Exp
    MAX = mybir.AluOpType.max
    MULT = mybir.AluOpType.mult

    with tile.TileContext(nc) as tc, ExitStack() as ctx:
        wpool = ctx.enter_context(tc.tile_pool(name="w", bufs=1))
        inp = ctx.enter_context(tc.tile_pool(name="inp", bufs=2))
        proj = ctx.enter_context(tc.tile_pool(name="proj", bufs=2))
        expp = ctx.enter_context(tc.tile_pool(name="expp", bufs=2))
        outp = ctx.enter_context(tc.tile_pool(name="outp", bufs=2))
        dpool = ctx.enter_context(tc.tile_pool(name="dinv", bufs=8))
        ps_qk = ctx.enter_context(tc.tile_pool(name="psqk", bufs=2, space="PSUM"))
        ps_sc = ctx.enter_context(tc.tile_pool(name="pssc", bufs=2, space="PSUM"))
        ps_v = ctx.enter_context(tc.tile_pool(name="psv", bufs=2, space="PSUM"))
        ps_o = ctx.enter_context(tc.tile_pool(name="pso", bufs=2, space="PSUM"))

        ones_col = wpool.tile([P, SCH, 2], f32, tag="ones_col")
        nc.gpsimd.memset(ones_col[:], 1.0)
        ebias_t = None
        if exp_bias != 0.0:
            ebias_t = wpool.tile([P, 1], f32, tag="ebias")
            nc.gpsimd.memset(ebias_t[:], exp_bias)

        wt = {}
        for cc in range(CYC):
            for name, wd in (("q", wq), ("k", wk), ("v", wv)):
                t = wpool.tile([P, ECH, E], mmdt, tag=f"w{name}{cc}")
                nc.scalar.dma_start(
                    out=t[:], in_=wd[cc].rearrange("(ch p) f -> p ch f", p=P))
                wt[name, cc] = t
        if with_bias:
            bt = {}
            for name, bd in (("q", bq), ("k", bk), ("v", bv)):
                for cc in range(CYC):
                    t = wpool.tile([1, E], mmdt, tag=f"b{name}{cc}")
                    nc.sync.dma_start(out=t[:], in_=bd[cc : cc + 1, :])
                    bt[name, cc] = t
            ones_f32 = wpool.tile([1, S], f32, tag="ones_f32")
            nc.gpsimd.memset(ones_f32[:], 1.0)
            ones_row = wpool.tile([1, S], mmdt, tag="ones")
            nc.vector.tensor_copy(ones_row[:], ones_f32[:])

        if pairs >= 4 and pairs % 2 == 0:
            batches = [(0, 1)] + [(i, 2) for i in range(1, pairs - 1, 2)] \
                + [(pairs - 1, 1)]
        else:
            batches = [(i, 1) for i in range(pairs)]
        for pb, PB in batches:
            qT_in = inp.tile([P, PB, ECH, S], mmdt, tag="qT_in")
            kT_in = inp.tile([P, PB, ECH, S], mmdt, tag="kT_in")
            vT_in = inp.tile([P, PB, ECH, S], mmdt, tag="vT_in")
            for t, d in ((qT_in, qt), (kT_in, kt), (vT_in, vt)):
                nc.sync.dma_start(
                    out=t[:],
                    in_=d[pb : pb + PB].rearrange("pp (ch p) s -> p pp ch s", p=P))
            outb = outp.tile([P, PB, SCH, E], f32, tag="outs")

            def stage_proj_qk(sub):
                p_idx = pb + sub
                cc = (p_idx * CYC) // pairs
                qTs = proj.tile([P, ECH, S], mmdt, tag="qTs")
                kTs = proj.tile([P, ECH, S], mmdt, tag="kTs")
                for name, srct, dst, scl in (("q", qT_in, qTs, SCALE),
                                             ("k", kT_in, kTs, 1.0)):
                    w = wt[name, cc]
                    for f in range(ECH):
                        ps = ps_qk.tile([P, S], f32, tag="ps_qk")
                        fsl = slice(f * P, (f + 1) * P)
                        for e in range(ECH):
                            nc.tensor.matmul(
                                ps[:], w[:, e, fsl], srct[:, sub, e, :],
                                start=(e == 0),
                                stop=(e == ECH - 1 and not with_bias))
                        if with_bias:
                            nc.tensor.matmul(
                                ps[:], bt[name, cc][:, fsl], ones_row[:],
                                start=False, stop=True)
                        if name == "q":
                            nc.scalar.activation(dst[:, f, :], ps[:], Relu,
                                                 scale=scl)
                        else:
                            nc.vector.tensor_scalar(
                                dst[:, f, :], ps[:], 0.0, None, MAX)
                return qTs, kTs

            def stage_v(sub):
                p_idx = pb + sub
                cc = (p_idx * CYC) // pairs
                vs = proj.tile([P, SCH, E + 2], mmdt, tag="vs")
                nc.vector.tensor_copy(vs[:, :, E : E + 2], ones_col[:])
                w = wt["v", cc]
                for t in range(SCH):
                    ps = ps_v.tile([P, E + 2], f32, tag="ps_v")
                    tsl = slice(t * P, (t + 1) * P)
                    for e in range(ECH):
                        nc.tensor.matmul(
                            ps[:, :E], vT_in[:, sub, e, tsl], w[:, e, :],
                            start=(e == 0),
                            stop=(e == ECH - 1 and not with_bias))
                    if with_bias:
                        nc.tensor.matmul(
                            ps[:, :E], ones_row[:, tsl], bt["v", cc][:],
                            start=False, stop=True)
                    nc.vector.tensor_scalar(
                        vs[:, t, :E], ps[:, :E], 0.0, None, MAX)
                return vs

            def stage_scores(sub, qTs, kTs):
                expTs = expp.tile([P, SCH, S], mmdt, tag="expTs")
                for t in range(SCH):
                    ps = ps_sc.tile([P, S], f32, tag="ps_sc")
                    tsl = slice(t * P, (t + 1) * P)
                    for f in range(ECH):
                        nc.tensor.matmul(
                            ps[:], kTs[:, f, tsl], qTs[:, f, :],
                            start=(f == 0), stop=(f == ECH - 1))
                    if ebias_t is None:
                        nc.scalar.activation(expTs[:, t, :], ps[:], Exp)
                    else:
                        nc.scalar.activation(expTs[:, t, :], ps[:], Exp,
                                             bias=ebias_t[:])
                return expTs

            def stage_out(sub, expTs, vs):
                for s in range(SCH):
                    ps = ps_o.tile([P, E + 2], f32, tag="ps_o")
                    ssl = slice(s * P, (s + 1) * P)
                    for t in range(SCH):
                        nc.tensor.matmul(
                            ps[:], expTs[:, t, ssl], vs[:, t, :],
                            start=(t == 0), stop=(t == SCH - 1))
                    dinv = dpool.tile([P, 1], f32, tag="dinv")
                    nc.vector.reciprocal(dinv[:], ps[:, E : E + 1])
                    nc.vector.tensor_scalar(
                        outb[:, sub, s, :], ps[:, :E], dinv[:], None, MULT)

            for sub in range(PB):
                qTs, kTs = stage_proj_qk(sub)
                vs = stage_v(sub)
                expTs = stage_scores(sub, qTs, kTs)
                stage_out(sub, expTs, vs)

            for sub in range(PB):
                nc.scalar.dma_start(
                    out=out[pb + sub].rearrange("(sch p) e -> p sch e", p=P),
                    in_=outb[:, sub])

    nc.compile()
    return nc


_BUILT = {}


def _get_built(pairs=PAIRS_FULL, with_bias=False, mm_dtype=MM_DTYPE):
    key = (pairs, with_bias, mm_dtype)
    if key not in _BUILT:
        if mm_dtype == "dr8":
            assert not with_bias
            _BUILT[key] = _build_dr(pairs, proj_dtype="f8")
        elif mm_dtype == "mix8":
            assert not with_bias
            _BUILT[key] = _build_dr(pairs, proj_dtype="f16", attn_dtype="f8")
        elif mm_dtype == "f16v4":
            assert not with_bias
            _BUILT[key] = _build_dr(pairs, proj_dtype="f16", attn_dtype="f16")
        else:
            _BUILT[key] = _build_f16(pairs, with_bias, mm_dtype)
    return _BUILT[key]


def _round_fp32r(a):
    a = np.ascontiguousarray(a, dtype=np.float32)
    u = a.view(np.uint32).copy()
    u += np.uint32(0x800)
    u &= np.uint32(0xFFFFF000)
    return u.view(np.float32)


def _converter(mm_dtype):
    if mm_dtype == "f32r":
        return _round_fp32r
    if mm_dtype == "f16":
        return lambda x: np.ascontiguousarray(x, np.float16)
    if mm_dtype == "bf16":
        import ml_dtypes
        return lambda x: np.ascontiguousarray(
            np.asarray(x, np.float32).astype(ml_dtypes.bfloat16))
    if mm_dtype == "dr8":
        import ml_dtypes
        return lambda x: np.ascontiguousarray(
            np.asarray(x, np.float32).astype(ml_dtypes.float8_e4m3))
    if mm_dtype in ("mix8", "f16v4"):
        return lambda x: np.ascontiguousarray(x, np.float16)
    return lambda x: np.ascontiguousarray(x, np.float32)


def _shard_inputs(query, key, value, wq, wk, wv, bq, bk, bv, with_bias,
                  mm_dtype=MM_DTYPE):
    """Per-core input maps. Core m owns cycles [m*CYC, (m+1)*CYC)."""
    r = _converter(mm_dtype)
    in_maps = []
    for m in range(N_CORES):
        cs = slice(m * CYC, (m + 1) * CYC)
        im = {
            # [B, CYC, S, E] -> [pairs(cc-major), E, S] (host-side transpose)
            "qt": r(query[:, cs].transpose(1, 0, 3, 2)).reshape(PAIRS_FULL, E, S),
            "kt": r(key[:, cs].transpose(1, 0, 3, 2)).reshape(PAIRS_FULL, E, S),
            "vt": r(value[:, cs].transpose(1, 0, 3, 2)).reshape(PAIRS_FULL, E, S),
            "wq": r(wq[cs]),
            "wk": r(wk[cs]),
            "wv": r(wv[cs]),
        }
        if with_bias:
            im["bq"] = r(bq[cs, 0])
            im["bk"] = r(bk[cs, 0])
            im["bv"] = r(bv[cs, 0])
        in_maps.append(im)
    return in_maps


def kernel(**inputs):
    from concourse.bass_utils import run_bass_kernel_spmd

    query = np.asarray(inputs["query"], dtype=np.float32)
    key = np.asarray(inputs["key"], dtype=np.float32)
    value = np.asarray(inputs["value"], dtype=np.float32)
    wq = np.asarray(inputs["q_proj_weight"], dtype=np.float32)
    wk = np.asarray(inputs["k_proj_weight"], dtype=np.float32)
    wv = np.asarray(inputs["v_proj_weight"], dtype=np.float32)
    bq = np.asarray(inputs["q_proj_bias"], dtype=np.float32)
    bk = np.asarray(inputs["k_proj_bias"], dtype=np.float32)
    bv = np.asarray(inputs["v_proj_bias"], dtype=np.float32)

    with_bias = bool(np.any(bq) or np.any(bk) or np.any(bv))
    mm_dtype = "f16" if with_bias else MM_DTYPE
    nc = _get_built(PAIRS_FULL, with_bias, mm_dtype)
    in_maps = _shard_inputs(query, key, value, wq, wk, wv, bq, bk, bv,
                            with_bias, mm_dtype)

    res = None
    for attempt in range(3):
        try:
            res = run_bass_kernel_spmd(nc, in_maps, list(range(N_CORES)))
            break
        except Exception:
            if attempt == 2:
                raise
    out = np.empty((B, C, S, E), dtype=np.float32)
    for m in range(N_CORES):
        o = res.results[m]["out"].reshape(CYC, B, S, E)
        out[:, m * CYC : (m + 1) * CYC] = o.transpose(1, 0, 2, 3)
    return out


if __name__ == "__main__":
    rng = np.random.default_rng(0)
    ins = {
        "query": rng.standard_normal((B, C, S, E), dtype=np.float32),
        "key": rng.standard_normal((B, C, S, E), dtype=np.float32),
        "value": rng.standard_normal((B, C, S, E), dtype=np.float32),
        "q_proj_weight": rng.standard_normal((C, E, E), dtype=np.float32) * 0.0625,
        "k_proj_weight": rng.standard_normal((C, E, E), dtype=np.float32) * 0.0625,
        "v_proj_weight": rng.standard_normal((C, E, E), dtype=np.float32) * 0.0625,
        "q_proj_bias": np.zeros((C, 1, E), np.float32),
        "k_proj_bias": np.zeros((C, 1, E), np.float32),
        "v_proj_bias": np.zeros((C, 1, E), np.float32),
    }
    o = kernel(**ins)
    print("out", o.shape, o.dtype, float(np.abs(o).max()))


# revision 23
# speedup vs baseline: 1.1883x; 1.0121x over previous
"""Trainium2 Bass kernel for CycleWiseSelfAttention.

Problem: B=8, C=16, S=512, E=256 (fp32)
    q = relu(query @ Wq[c] + bq[c]) * E**-0.5
    k = relu(key   @ Wk[c] + bk[c])
    v = relu(value @ Wv[c] + bv[c])
    out = softmax(q @ k^T, axis=-1) @ v        (per (b, c) pair)

Sharding: cycle-parallel across 8 cores (2 cycles per core, all 8 batches).
Each core handles 16 independent (b, c) attention problems; per-cycle weights
go only to their owning core. No collectives.

Default build ("dr8"): all five matmul groups run as fp8e4 (e4m3) DoubleRow
matmuls — each instruction contracts K=256 (two 128-row halves packed per PE
cell), halving the PE streaming cycles vs fp16.  PSUM is one persistent
8-bank tensor with fixed per-pair roles:
    banks 0-1: q proj (f0,f1)  -> reused by scores t0,t1
    banks 2-3: k proj (f0,f1)  -> reused by scores t2,t3
    banks 4-7: v proj (t0..t3) -> reused by attn-out s0..s3
Fixed roles let every drain be one big cross-bank op:
    qrelu/krelu: one vector op over 2 banks each -> fp8 SBUF
    vrelu:       one vector op over 4 banks      -> fp8 SBUF
    exp:         ONE scalar activation over banks 0-3 (2048 el/partition),
                 with the softmax scale E**-0.5 and a -2 bias folded in
                 (softmax-invariant shift keeps fp8 exp outputs in range)
    norm:        scalar activation Copy with per-partition scale = 1/denom
The denominator comes from ones-columns appended to v (cols 256.. of a
272-wide v tile; 272 keeps the DoubleRow rhs stride a multiple of 16), so
attn-out computes [out_unnorm | denom] in one pass.

The f16 path (old baseline) is kept for the with_bias fallback and A/B runs.
"""

import numpy as np

B, C, S, E = 8, 16, 512, 256
N_CORES = 8
CYC = C // N_CORES          # cycles per core = 2
PAIRS_FULL = B * CYC        # (b, c) pairs per core = 16
P = 128
ECH = E // P                # e/f chunks = 2
SCH = S // P                # s/t chunks = 4
VF = 272                    # v free dim: 256 data + 16 ones cols (stride %16==0)
SCALE = float(E) ** -0.5
EXP_BIAS = -2.0             # softmax-invariant shift; keeps fp8 exp in range
MM_DTYPE = "mix8"


def _build_dr(pairs=PAIRS_FULL, proj_dtype="f8", attn_dtype="f8"):
    """Pipelined build (no bias support).

    proj_dtype="f16" keeps the three projection matmuls (and their DRAM
    inputs/weights) in fp16 for accuracy; attn_dtype picks fp8 DoubleRow
    ("f8") or plain fp16 ("f16") for the two attention matmul groups.
    """
    import concourse.bass as bass  # noqa: F401
    import concourse.bacc as bacc
    import concourse.tile as tile
    from concourse import mybir
    from contextlib import ExitStack

    f32 = mybir.dt.float32
    f8 = mybir.dt.float8e4
    f16 = mybir.dt.float16
    pdt = f8 if proj_dtype == "f8" else f16
    adt = f8 if attn_dtype == "f8" else f16
    vf = VF if attn_dtype == "f8" else E + 2
    DR = mybir.MatmulPerfMode.DoubleRow

    nc = bacc.Bacc("TRN2", target_bir_lowering=False, debug=False,
                   num_devices=N_CORES)

    qt = nc.dram_tensor("qt", [pairs, E, S], pdt, kind="ExternalInput").ap()
    kt = nc.dram_tensor("kt", [pairs, E, S], pdt, kind="ExternalInput").ap()
    vt = nc.dram_tensor("vt", [pairs, E, S], pdt, kind="ExternalInput").ap()
    wq = nc.dram_tensor("wq", [CYC, E, E], pdt, kind="ExternalInput").ap()
    wk = nc.dram_tensor("wk", [CYC, E, E], pdt, kind="ExternalInput").ap()
    wv = nc.dram_tensor("wv", [CYC, E, E], pdt, kind="ExternalInput").ap()
    out = nc.dram_tensor("out", [pairs, S, E], f32, kind="ExternalOutput").ap()

    Relu = mybir.ActivationFunctionType.Relu
    Exp = mybir.ActivationFunctionType.Exp
    Copy = mybir.ActivationFunctionType.Copy
    MAX = mybir.AluOpType.max
    MULT = mybir.AluOpType.mult

    def mm(ps, lhsT, rhs, start=True, stop=True):
        nc.tensor.matmul(ps, lhsT, rhs, start=start, stop=stop, perf_mode=DR)

    # engine per drain site: "v" = vector, "s" = scalar
    # keep Scalar's in-order queue clear of proj drains so the chain-critical
    # exp ops issue as early as possible; norms are tail work
    ENG_QRELU = "v"
    ENG_KRELU = "s"
    ENG_VRELU = "v"
    ENG_NORM = ("s", "v", "v", "v")   # per s-chunk

    def relu_drain(eng, dst, src):
        if eng == "v":
            nc.vector.tensor_scalar(dst, src, 0.0, None, MAX)
        else:
            nc.scalar.activation(dst, src, Relu)

    with tile.TileContext(nc) as tc, ExitStack() as ctx:
        wpool = ctx.enter_context(tc.tile_pool(name="w", bufs=1))
        inp = ctx.enter_context(tc.tile_pool(name="inp", bufs=2))
        proj = ctx.enter_context(tc.tile_pool(name="proj", bufs=2))
        expp = ctx.enter_context(tc.tile_pool(name="expp", bufs=2))
        outp = ctx.enter_context(tc.tile_pool(name="outp", bufs=2))
        dpool = ctx.enter_context(tc.tile_pool(name="dinv", bufs=8))
        # Two PSUM streams ("ps0"/"ps1"), each 2 bufs x 2 banks = 4 banks;
        # pairs alternate streams so stalls in one are covered by the other.
        psp = ctx.enter_context(tc.tile_pool(name="ps", bufs=2, space="PSUM"))

        ebias = wpool.tile([P, 1], f32, tag="ebias")
        nc.gpsimd.memset(ebias[:], EXP_BIAS)

        # persistent v tiles (2, alternating per pair); ones cols preset
        vs_t = []
        for i in range(2):
            t = wpool.tile([P, SCH, vf], adt, tag=f"vs{i}")
            nc.gpsimd.memset(t[:], 1.0)
            vs_t.append(t)

        # persistent weights [128, ech, E] per (proj, cycle), scalar ring.
        # Only cycle-0 weights load upfront (pairs are cc-major, so cycle-1
        # weights are not needed until halfway); cycle-1 loads are emitted
        # after the first couple to keep them off the startup critical path.
        wsrc = {"q": wq, "k": wk, "v": wv}
        wt = {}
        for name in ("q", "k", "v"):
            for cc in range(CYC):
                wt[name, cc] = wpool.tile([P, ECH, E], pdt, tag=f"w{name}{cc}",
                                          name=f"w{name}{cc}")
        def load_weights(cc, ring):
            for name in ("q", "k", "v"):
                ring.dma_start(
                    out=wt[name, cc][:],
                    in_=wsrc[name][cc].rearrange("(ch p) f -> p ch f", p=P))
        for cc in range(CYC):
            load_weights(cc, nc.scalar)

        # Dual-stream emission: pairs 2i and 2i+1 advance in lockstep with
        # separate 4-bank PSUM sets ("psA"/"psB"), interleaved stage by
        # stage, so either stream's dependency stalls are covered by the
        # other stream's ready work.
        def ps_tile(p_idx):
            return psp.tile([P, 2, S], f32, tag=f"ps{p_idx % 2}",
                            name=f"psT{p_idx % 2}")

        def st_load(pb, PB):
            qT_in = inp.tile([P, PB, ECH, S], pdt, tag="qT_in")
            kT_in = inp.tile([P, PB, ECH, S], pdt, tag="kT_in")
            vT_in = inp.tile([P, PB, ECH, S], pdt, tag="vT_in")
            for t, d in ((qT_in, qt), (kT_in, kt), (vT_in, vt)):
                nc.sync.dma_start(
                    out=t[:],
                    in_=d[pb : pb + PB].rearrange("pp (ch p) s -> p pp ch s", p=P))
            outb = outp.tile([P, PB, SCH, E], f32, tag="outs")
            return qT_in, kT_in, vT_in, outb

        def pair_cc(p_idx):
            return p_idx % CYC

        def st_qk(p_idx, sub, srct, wname, eng):
            cc = pair_cc(p_idx)
            T = ps_tile(p_idx)
            for f in range(ECH):
                fsl = slice(f * P, (f + 1) * P)
                if proj_dtype == "f8":
                    mm(T[:, f, :], wt[wname, cc][:, :, fsl], srct[:, sub])
                else:
                    for e in range(ECH):
                        nc.tensor.matmul(
                            T[:, f, :], wt[wname, cc][:, e, fsl],
                            srct[:, sub, e, :],
                            start=(e == 0), stop=(e == ECH - 1))
            dst = proj.tile([P, ECH, S], adt, tag=wname + "Ts",
                            name=wname + "Ts")
            relu_drain(eng, dst[:], T[:])
            return dst

        def st_v(p_idx, sub, vT_in):
            cc = pair_cc(p_idx)
            vs = vs_t[p_idx % 2]
            T = ps_tile(p_idx)
            for t in range(SCH):
                tsl = slice(t * P, (t + 1) * P)
                osl = slice((t % 2) * E, (t % 2) * E + E)
                if proj_dtype == "f8":
                    mm(T[:, t // 2, osl], vT_in[:, sub, :, tsl],
                       wt["v", cc][:])
                else:
                    for e in range(ECH):
                        nc.tensor.matmul(
                            T[:, t // 2, osl], vT_in[:, sub, e, tsl],
                            wt["v", cc][:, e, :],
                            start=(e == 0), stop=(e == ECH - 1))
            relu_drain(
                ENG_VRELU,
                vs[:, :, :E].rearrange("p (b c) x -> p b c x", b=2),
                T[:].rearrange("p b (c x) -> p b c x", x=E))
            return vs

        def st_sc(p_idx, g, qTs, kTs, expTs):
            T = ps_tile(p_idx)
            for i in range(2):
                t = 2 * g + i
                tsl = slice(t * P, (t + 1) * P)
                if attn_dtype == "f8":
                    mm(T[:, i, :], kTs[:, :, tsl], qTs[:])
                else:
                    for f in range(ECH):
                        nc.tensor.matmul(
                            T[:, i, :], kTs[:, f, tsl], qTs[:, f, :],
                            start=(f == 0), stop=(f == ECH - 1))
            nc.scalar.activation(expTs[:, 2 * g : 2 * g + 2, :], T[:], Exp,
                                 bias=ebias[:], scale=SCALE)

        def st_o(p_idx, sub, sg, expTs, vs, outb):
            T = ps_tile(p_idx)
            for si in range(2):
                s = sg * 2 + si
                ssl = slice(s * P, (s + 1) * P)
                if attn_dtype == "f8":
                    for i in range(2):
                        mm(T[:, si, :vf],
                           expTs[:, 2 * i : 2 * i + 2, ssl],
                           vs[:, 2 * i : 2 * i + 2, :],
                           start=(i == 0), stop=(i == 1))
                else:
                    for t in range(SCH):
                        nc.tensor.matmul(
                            T[:, si, :vf], expTs[:, t, ssl], vs[:, t, :],
                            start=(t == 0), stop=(t == SCH - 1))
            dinv = dpool.tile([P, 2], f32, tag="dinv")
            nc.vector.reciprocal(dinv[:], T[:, :, E : E + 1])
            for si in range(2):
                s = sg * 2 + si
                if ENG_NORM[s] == "s":
                    nc.scalar.activation(
                        outb[:, sub, s, :], T[:, si, :E], Copy,
                        scale=dinv[:, si : si + 1])
                else:
                    nc.vector.tensor_scalar(
                        outb[:, sub, s, :], T[:, si, :E],
                        dinv[:, si : si + 1], None, MULT)

        # single-pair first batches shrink the head (first matmul waits on a
        # 0.5 MB load, not 3 MB); single-pair last batches shrink the store
        # tail the same way
        if pairs >= 6 and pairs % 2 == 0:
            couples = [(0, 1), (1, 1)] \
                + [(pb, 2) for pb in range(2, pairs - 2, 2)] \
                + [(pairs - 2, 1), (pairs - 1, 1)]
        else:
            couples = [(pb, 1) for pb in range(pairs)]
        for pb, PB in couples:
            qT_in, kT_in, vT_in, outb = st_load(pb, PB)
            subs = list(range(PB))
            qTs = {}; kTs = {}; vsd = {}; expd = {}
            for sub in subs:
                qTs[sub] = st_qk(pb + sub, sub, qT_in, "q", ENG_QRELU)
            for sub in subs:
                kTs[sub] = st_qk(pb + sub, sub, kT_in, "k", ENG_KRELU)
            for sub in subs:
                vsd[sub] = st_v(pb + sub, sub, vT_in)
            for sub in subs:
                expd[sub] = expp.tile([P, SCH, S], adt, tag="expTs",
                                      name="expTs")
            for g in range(2):
                for sub in subs:
                    st_sc(pb + sub, g, qTs[sub], kTs[sub], expd[sub])
            for sg in range(2):
                for sub in subs:
                    st_o(pb + sub, sub, sg, expd[sub], vsd[sub], outb)
            nc.gpsimd.dma_start(
                out=out[pb : pb + PB].rearrange(
                    "pp (sch p) e -> p pp sch e", p=P),
                in_=outb[:])

    nc.compile()
    return nc


def _build_f16(pairs=PAIRS_FULL, with_bias=False, mm_dtype="f16"):
    """Old fp16/f32r build (supports bias); kept as fallback."""
    import concourse.bass as bass  # noqa: F401
    import concourse.bacc as bacc
    import concourse.tile as tile
    from concourse import mybir
    from contextlib import ExitStack

    f32 = mybir.dt.float32
    mmdt = {"f32r": mybir.dt.float32r, "f32": mybir.dt.float32,
            "f16": mybir.dt.float16, "bf16": mybir.dt.bfloat16}[mm_dtype]
    exp_bias = -2.0 if mm_dtype in ("f16", "bf16") else 0.0

    nc = bacc.Bacc("TRN2", target_bir_lowering=False, debug=False,
                   num_devices=N_CORES)

    qt = nc.dram_tensor("qt", [pairs, E, S], mmdt, kind="ExternalInput").ap()
    kt = nc.dram_tensor("kt", [pairs, E, S], mmdt, kind="ExternalInput").ap()
    vt = nc.dram_tensor("vt", [pairs, E, S], mmdt, kind="ExternalInput").ap()
    wq = nc.dram_tensor("wq", [CYC, E, E], mmdt, kind="ExternalInput").ap()
    wk = nc.dram_tensor("wk", [CYC, E, E], mmdt, kind="ExternalInput").ap()
    wv = nc.dram_tensor("wv", [CYC, E, E], mmdt, kind="ExternalInput").ap()
    if with_bias:
        bq = nc.dram_tensor("bq", [CYC, E], mmdt, kind="ExternalInput").ap()
        bk = nc.dram_tensor("bk", [CYC, E], mmdt, kind="ExternalInput").ap()
        bv = nc.dram_tensor("bv", [CYC, E], mmdt, kind="ExternalInput").ap()
    out = nc.dram_tensor("out", [pairs, S, E], f32, kind="ExternalOutput").ap()

    Relu = mybir.ActivationFunctionType.Relu  # noqa: F841
    Exp = mybir.ActivationFunctionType.Exp
    MAX = mybir.AluOpType.max
    MULT = mybir.AluOpType.mult

    with tile.TileContext(nc) as tc, ExitStack() as ctx:
        wpool = ctx.enter_context(tc.tile_pool(name="w", bufs=1))
        inp = ctx.enter_context(tc.tile_pool(name="inp", bufs=2))
        proj = ctx.enter_context(tc.tile_pool(name="proj", bufs=2))
        expp = ctx.enter_context(tc.tile_pool(name="expp", bufs=2))
        outp = ctx.enter_context(tc.tile_pool(name="outp", bufs=2))
        dpool = ctx.enter_context(tc.tile_pool(name="dinv", bufs=8))
        ps_qk = ctx.enter_context(tc.tile_pool(name="psqk", bufs=2, space="PSUM"))
        ps_sc = ctx.enter_context(tc.tile_pool(name="pssc", bufs=2, space="PSUM"))
        ps_v = ctx.enter_context(tc.tile_pool(name="psv", bufs=2, space="PSUM"))
        ps_o = ctx.enter_context(tc.tile_pool(name="pso", bufs=2, space="PSUM"))

        ones_col = wpool.tile([P, SCH, 2], f32, tag="ones_col")
        nc.gpsimd.memset(ones_col[:], 1.0)
        ebias_t = None
        if exp_bias != 0.0:
            ebias_t = wpool.tile([P, 1], f32, tag="ebias")
            nc.gpsimd.memset(ebias_t[:], exp_bias)

        wt = {}
        for cc in range(CYC):
            for name, wd in (("q", wq), ("k", wk), ("v", wv)):
                t = wpool.tile([P, ECH, E], mmdt, tag=f"w{name}{cc}")
                nc.scalar.dma_start(
                    out=t[:], in_=wd[cc].rearrange("(ch p) f -> p ch f", p=P))
                wt[name, cc] = t
        if with_bias:
            bt = {}
            for name, bd in (("q", bq), ("k", bk), ("v", bv)):
                for cc in range(CYC):
                    t = wpool.tile([1, E], mmdt, tag=f"b{name}{cc}")
                    nc.sync.dma_start(out=t[:], in_=bd[cc : cc + 1, :])
                    bt[name, cc] = t
            ones_f32 = wpool.tile([1, S], f32, tag="ones_f32")
            nc.gpsimd.memset(ones_f32[:], 1.0)
            ones_row = wpool.tile([1, S], mmdt, tag="ones")
            nc.vector.tensor_copy(ones_row[:], ones_f32[:])

        if pairs >= 4 and pairs % 2 == 0:
            batches = [(0, 1)] + [(i, 2) for i in range(1, pairs - 1, 2)] \
                + [(pairs - 1, 1)]
        else:
            batches = [(i, 1) for i in range(pairs)]
        for pb, PB in batches:
            qT_in = inp.tile([P, PB, ECH, S], mmdt, tag="qT_in")
            kT_in = inp.tile([P, PB, ECH, S], mmdt, tag="kT_in")
            vT_in = inp.tile([P, PB, ECH, S], mmdt, tag="vT_in")
            for t, d in ((qT_in, qt), (kT_in, kt), (vT_in, vt)):
                nc.sync.dma_start(
                    out=t[:],
                    in_=d[pb : pb + PB].rearrange("pp (ch p) s -> p pp ch s", p=P))
            outb = outp.tile([P, PB, SCH, E], f32, tag="outs")

            def stage_proj_qk(sub):
                p_idx = pb + sub
                cc = p_idx % CYC
                qTs = proj.tile([P, ECH, S], mmdt, tag="qTs")
                kTs = proj.tile([P, ECH, S], mmdt, tag="kTs")
                for name, srct, dst, scl in (("q", qT_in, qTs, SCALE),
                                             ("k", kT_in, kTs, 1.0)):
                    w = wt[name, cc]
                    for f in range(ECH):
                        ps = ps_qk.tile([P, S], f32, tag="ps_qk")
                        fsl = slice(f * P, (f + 1) * P)
                        for e in range(ECH):
                            nc.tensor.matmul(
                                ps[:], w[:, e, fsl], srct[:, sub, e, :],
                                start=(e == 0),
                                stop=(e == ECH - 1 and not with_bias))
                        if with_bias:
                            nc.tensor.matmul(
                                ps[:], bt[name, cc][:, fsl], ones_row[:],
                                start=False, stop=True)
                        if name == "q":
                            nc.scalar.activation(dst[:, f, :], ps[:], Relu,
                                                 scale=scl)
                        else:
                            nc.vector.tensor_scalar(
                                dst[:, f, :], ps[:], 0.0, None, MAX)
                return qTs, kTs

            def stage_v(sub):
                p_idx = pb + sub
                cc = p_idx % CYC
                vs = proj.tile([P, SCH, E + 2], mmdt, tag="vs")
                nc.vector.tensor_copy(vs[:, :, E : E + 2], ones_col[:])
                w = wt["v", cc]
                for t in range(SCH):
                    ps = ps_v.tile([P, E + 2], f32, tag="ps_v")
                    tsl = slice(t * P, (t + 1) * P)
                    for e in range(ECH):
                        nc.tensor.matmul(
                            ps[:, :E], vT_in[:, sub, e, tsl], w[:, e, :],
                            start=(e == 0),
                            stop=(e == ECH - 1 and not with_bias))
                    if with_bias:
                        nc.tensor.matmul(
                            ps[:, :E], ones_row[:, tsl], bt["v", cc][:],
                            start=False, stop=True)
                    nc.vector.tensor_scalar(
                        vs[:, t, :E], ps[:, :E], 0.0, None, MAX)
                return vs

            def stage_scores(sub, qTs, kTs):
                expTs = expp.tile([P, SCH, S], mmdt, tag="expTs")
                for t in range(SCH):
                    ps = ps_sc.tile([P, S], f32, tag="ps_sc")
                    tsl = slice(t * P, (t + 1) * P)
                    for f in range(ECH):
                        nc.tensor.matmul(
                            ps[:], kTs[:, f, tsl], qTs[:, f, :],
                            start=(f == 0), stop=(f == ECH - 1))
                    if ebias_t is None:
                        nc.scalar.activation(expTs[:, t, :], ps[:], Exp)
                    else:
                        nc.scalar.activation(expTs[:, t, :], ps[:], Exp,
                                             bias=ebias_t[:])
                return expTs

            def stage_out(sub, expTs, vs):
                for s in range(SCH):
                    ps = ps_o.tile([P, E + 2], f32, tag="ps_o")
                    ssl = slice(s * P, (s + 1) * P)
                    for t in range(SCH):
                        nc.tensor.matmul(
                            ps[:], expTs[:, t, ssl], vs[:, t, :],
                            start=(t == 0), stop=(t == SCH - 1))
                    dinv = dpool.tile([P, 1], f32, tag="dinv")
                    nc.vector.reciprocal(dinv[:], ps[:, E : E + 1])
                    nc.vector.tensor_scalar(
                        outb[:, sub, s, :], ps[:, :E], dinv[:], None, MULT)

            for sub in range(PB):
                qTs, kTs = stage_proj_qk(sub)
                vs = stage_v(sub)
                expTs = stage_scores(sub, qTs, kTs)
                stage_out(sub, expTs, vs)

            for sub in range(PB):
                nc.scalar.dma_start(
                    out=out[pb + sub].rearrange("(sch p) e -> p sch e", p=P),
                    in_=outb[:, sub])

    nc.compile()
    return nc


_BUILT = {}


def _get_built(pairs=PAIRS_FULL, with_bias=False, mm_dtype=MM_DTYPE):
    key = (pairs, with_bias, mm_dtype)
    if key not in _BUILT:
        if mm_dtype == "dr8":
            assert not with_bias
            _BUILT[key] = _build_dr(pairs, proj_dtype="f8")
        elif mm_dtype == "mix8":
            assert not with_bias
            _BUILT[key] = _build_dr(pairs, proj_dtype="f16", attn_dtype="f8")
        elif mm_dtype == "f16v4":
            assert not with_bias
            _BUILT[key] = _build_dr(pairs, proj_dtype="f16", attn_dtype="f16")
        else:
            _BUILT[key] = _build_f16(pairs, with_bias, mm_dtype)
    return _BUILT[key]


def _round_fp32r(a):
    a = np.ascontiguousarray(a, dtype=np.float32)
    u = a.view(np.uint32).copy()
    u += np.uint32(0x800)
    u &= np.uint32(0xFFFFF000)
    return u.view(np.float32)


def _converter(mm_dtype):
    if mm_dtype == "f32r":
        return _round_fp32r
    if mm_dtype == "f16":
        return lambda x: np.ascontiguousarray(x, np.float16)
    if mm_dtype == "bf16":
        import ml_dtypes
        return lambda x: np.ascontiguousarray(
            np.asarray(x, np.float32).astype(ml_dtypes.bfloat16))
    if mm_dtype == "dr8":
        import ml_dtypes
        return lambda x: np.ascontiguousarray(
            np.asarray(x, np.float32).astype(ml_dtypes.float8_e4m3))
    if mm_dtype in ("mix8", "f16v4"):
        return lambda x: np.ascontiguousarray(x, np.float16)
    return lambda x: np.ascontiguousarray(x, np.float32)


def _shard_inputs(query, key, value, wq, wk, wv, bq, bk, bv, with_bias,
                  mm_dtype=MM_DTYPE):
    """Per-core input maps. Core m owns cycles [m*CYC, (m+1)*CYC)."""
    r = _converter(mm_dtype)
    in_maps = []
    for m in range(N_CORES):
        cs = slice(m * CYC, (m + 1) * CYC)
        im = {
            # [B, CYC, S, E] -> [pairs, E, S] (host-side transpose)
            "qt": r(query[:, cs].transpose(0, 1, 3, 2)).reshape(PAIRS_FULL, E, S),
            "kt": r(key[:, cs].transpose(0, 1, 3, 2)).reshape(PAIRS_FULL, E, S),
            "vt": r(value[:, cs].transpose(0, 1, 3, 2)).reshape(PAIRS_FULL, E, S),
            "wq": r(wq[cs]),
            "wk": r(wk[cs]),
            "wv": r(wv[cs]),
        }
        if with_bias:
            im["bq"] = r(bq[cs, 0])
            im["bk"] = r(bk[cs, 0])
            im["bv"] = r(bv[cs, 0])
        in_maps.append(im)
    return in_maps


def kernel(**inputs):
    from concourse.bass_utils import run_bass_kernel_spmd

    query = np.asarray(inputs["query"], dtype=np.float32)
    key = np.asarray(inputs["key"], dtype=np.float32)
    value = np.asarray(inputs["value"], dtype=np.float32)
    wq = np.asarray(inputs["q_proj_weight"], dtype=np.float32)
    wk = np.asarray(inputs["k_proj_weight"], dtype=np.float32)
    wv = np.asarray(inputs["v_proj_weight"], dtype=np.float32)
    bq = np.asarray(inputs["q_proj_bias"], dtype=np.float32)
    bk = np.asarray(inputs["k_proj_bias"], dtype=np.float32)
    bv = np.asarray(inputs["v_proj_bias"], dtype=np.float32)

    with_bias = bool(np.any(bq) or np.any(bk) or np.any(bv))
    mm_dtype = "f16" if with_bias else MM_DTYPE
    nc = _get_built(PAIRS_FULL, with_bias, mm_dtype)
    in_maps = _shard_inputs(query, key, value, wq, wk, wv, bq, bk, bv,
                            with_bias, mm_dtype)

    res = None
    for attempt in range(3):
        try:
            res = run_bass_kernel_spmd(nc, in_maps, list(range(N_CORES)))
            break
        except Exception:
            if attempt == 2:
                raise
    out = np.empty((B, C, S, E), dtype=np.float32)
    for m in range(N_CORES):
        o = res.results[m]["out"].reshape(B, CYC, S, E)
        out[:, m * CYC : (m + 1) * CYC] = o
    return out


if __name__ == "__main__":
    rng = np.random.default_rng(0)
    ins = {
        "query": rng.standard_normal((B, C, S, E), dtype=np.float32),
        "key": rng.standard_normal((B, C, S, E), dtype=np.float32),
        "value": rng.standard_normal((B, C, S, E), dtype=np.float32),
        "q_proj_weight": rng.standard_normal((C, E, E), dtype=np.float32) * 0.0625,
        "k_proj_weight": rng.standard_normal((C, E, E), dtype=np.float32) * 0.0625,
        "v_proj_weight": rng.standard_normal((C, E, E), dtype=np.float32) * 0.0625,
        "q_proj_bias": np.zeros((C, 1, E), np.float32),
        "k_proj_bias": np.zeros((C, 1, E), np.float32),
        "v_proj_bias": np.zeros((C, 1, E), np.float32),
    }
    o = kernel(**ins)
    print("out", o.shape, o.dtype, float(np.abs(o).max()))
